# revision 11
# baseline (speedup 1.0000x reference)
"""Guide-token attention kernel for Trainium2 (8 NeuronCores).

Module: y[b] = softmax(((Q+tQ) @ (K+tK)^T)/sqrt(hd)) @ V  per head, where
  Q = x @ Wq^T + bq, K = x @ Wk^T + bk, V = x @ Wv^T + bv,
  tQ/tK are the projections of a per-batch guide token (broadcast over seq).

Shapes: x [4, 1024, 1024], tokens [4, 1, 1024], W* [1024, 1024], b* [1024].
H=16 heads, hd=64.

Sharding: 8 cores = 4 batches x 2 head-groups (8 heads each). Weights are
column-sharded per head group; each core sees one batch -> zero cross-core
communication.

Layout strategy (everything chosen so the PE contracts over the partition
axis with no on-chip transposes):
  - host pre-transposes x[b] -> xT [D, S] and W slices -> wT [D, 512], casts
    to bf16.
  - QT/KT computed transposed [feat, S]; V computed natural [S, feat].
  - guide-token adds + bias (tQ + 2*bq etc.) are tiny [512] vectors,
    precomputed on host, applied as per-partition scalars during PSUM
    eviction (Q side also pre-scaled by 1/sqrt(hd)).
  - scores computed directly transposed: sT[k, q] = cK @ cQ^T per head
    (lhsT = cKT slice, rhs = cQT slice, K=hd=64).
  - exp on ScalarE, writing bf16 probs (softmax max-subtraction skipped:
    |scores| <= ~15, exp safely in range).
  - AV: lhsT = V chunk [k,64] + appended ones column (-> row 64 of the
    output accumulates the softmax denominator), rhs = probsT [k, q],
    accumulated over k chunks -> outT [65, q] in PSUM.
  - normalization: reciprocal of denominator row, PE K=1 matmul broadcasts
    it across 64 partitions, one VectorE multiply -> yT [feat, S] fp32.
  - host reassembles y[b][:, cols] = yT^T and adds bv once at the end
    (attn rows sum to 1, so y = softmax@V0 + bv exactly).
"""

import os

import numpy as np
import ml_dtypes

import concourse.bass as bass
import concourse.tile as tile
from concourse import bacc
from concourse import mybir
from concourse.bass_utils import run_bass_kernel_spmd

B = 4
S = 1024
D = 1024
H = 16
HD = 64
NCORES = 8
FPG = 512          # features per head-group (8 heads * 64)
NKC = D // 128     # contraction chunks for projections
NFT = FPG // 128   # feature tiles per group
NST = S // 128     # sequence tiles
NQB = S // 512     # 512-wide query blocks
HPG = 8            # heads per group

BF16 = mybir.dt.bfloat16
F32 = mybir.dt.float32

_CACHE = {}


def _build(debug_dumps=False):
    nc = bacc.Bacc()

    xT = nc.declare_dram_parameter("xT", [D, S], BF16, isOutput=False)
    wqT = nc.declare_dram_parameter("wqT", [D, FPG], BF16, isOutput=False)
    wkT = nc.declare_dram_parameter("wkT", [D, FPG], BF16, isOutput=False)
    wvT = nc.declare_dram_parameter("wvT", [D, FPG], BF16, isOutput=False)
    qadd = nc.declare_dram_parameter("qadd", [128, NFT], F32, isOutput=False)
    kadd = nc.declare_dram_parameter("kadd", [128, NFT], F32, isOutput=False)
    yT = nc.declare_dram_parameter("yT", [FPG, S], F32, isOutput=True)
    dbg = {}
    if debug_dumps:
        dbg["d_cq"] = nc.declare_dram_parameter("d_cq", [128, S], BF16, isOutput=True)
        dbg["d_ck"] = nc.declare_dram_parameter("d_ck", [128, S], BF16, isOutput=True)
        dbg["d_v"] = nc.declare_dram_parameter("d_v", [128, HD + 1], BF16, isOutput=True)
        dbg["d_pr"] = nc.declare_dram_parameter("d_pr", [128, 512], BF16, isOutput=True)
        dbg["d_av"] = nc.declare_dram_parameter("d_av", [HD + 1, 512], F32, isOutput=True)
        dbg["d_rec"] = nc.declare_dram_parameter("d_rec", [1, 512], F32, isOutput=True)
        dbg["d_bc"] = nc.declare_dram_parameter("d_bc", [HD, 512], F32, isOutput=True)

    with tile.TileContext(nc) as tc:
        with (
            tc.tile_pool(name="persist", bufs=1) as persist,
            tc.tile_pool(name="probs", bufs=16) as probs_pool,
            tc.tile_pool(name="evict", bufs=4) as evict_pool,
            tc.tile_pool(name="psA", bufs=4, space=bass.MemorySpace.PSUM) as psA,
            tc.tile_pool(name="psAV", bufs=2, space=bass.MemorySpace.PSUM) as psAV,
            tc.tile_pool(name="psB", bufs=2, space=bass.MemorySpace.PSUM) as psB,
        ):
            # ---- persistent SBUF tensors ----
            xt = persist.tile([128, NKC, S], BF16)         # xT chunks
            wq = persist.tile([128, NKC, FPG], BF16)
            wk = persist.tile([128, NKC, FPG], BF16)
            wv = persist.tile([128, NKC, FPG], BF16)
            qa = persist.tile([128, NFT], F32)
            ka = persist.tile([128, NFT], F32)
            cq = persist.tile([128, NFT, S], BF16)         # cQT/8  [feat, S]
            ck = persist.tile([128, NFT, S], BF16)         # cKT    [feat, S]
            # V' [k, st, head, 65]: col 64 = ones (denominator trick)
            vt = persist.tile([128, NST, HPG, HD + 1], BF16)
            yt = persist.tile([128, NFT, S], F32)          # yT [feat, S]
            ones = persist.tile([1, HD], F32)

            # ---- input DMAs ----
            for kc in range(NKC):
                nc.sync.dma_start(out=wq[:, kc, :], in_=wqT[kc * 128:(kc + 1) * 128, :])
            for kc in range(NKC):
                nc.sync.dma_start(out=xt[:, kc, :], in_=xT[kc * 128:(kc + 1) * 128, :])
            for kc in range(NKC):
                nc.sync.dma_start(out=wk[:, kc, :], in_=wkT[kc * 128:(kc + 1) * 128, :])
            for kc in range(NKC):
                nc.sync.dma_start(out=wv[:, kc, :], in_=wvT[kc * 128:(kc + 1) * 128, :])
            nc.sync.dma_start(out=qa[:], in_=qadd[:])
            nc.sync.dma_start(out=ka[:], in_=kadd[:])

            nc.vector.memset(ones[:], 1.0)
            nc.vector.memset(vt[:, :, :, HD:HD + 1], 1.0)

            # ---- projections ----
            # QT/KT transposed: out[feat_tile, S_block] = sum_kc wT_chunk^T @ xT_chunk
            for name, w_sb, add_sb, scale in (
                ("q", wq, qa, 0.125),
                ("k", wk, ka, 1.0),
            ):
                dst = cq if name == "q" else ck
                for ft in range(NFT):
                    for sb in range(NQB):
                        acc = psA.tile([128, 512], F32, tag="psA")
                        for kc in range(NKC):
                            nc.tensor.matmul(
                                acc[:],
                                w_sb[:, kc, ft * 128:(ft + 1) * 128],
                                xt[:, kc, sb * 512:(sb + 1) * 512],
                                start=(kc == 0),
                                stop=(kc == NKC - 1),
                            )
                        nc.vector.tensor_scalar(
                            out=dst[:, ft, sb * 512:(sb + 1) * 512],
                            in0=acc[:],
                            scalar1=scale,
                            scalar2=add_sb[:, ft:ft + 1],
                            op0=mybir.AluOpType.mult,
                            op1=mybir.AluOpType.add,
                        )

            # V natural layout: out[S_tile, feat] = sum_kc xT_chunk_slice^T @ wvT_chunk
            for st in range(NST):
                acc = psA.tile([128, 512], F32, tag="psA")
                for kc in range(NKC):
                    nc.tensor.matmul(
                        acc[:],
                        xt[:, kc, st * 128:(st + 1) * 128],
                        wv[:, kc, :],
                        start=(kc == 0),
                        stop=(kc == NKC - 1),
                    )
                nc.vector.tensor_copy(
                    out=vt[:, st, :, 0:HD],
                    in_=acc[:],
                )

            # ---- attention (per head, per 512-wide query block) ----
            for h in range(HPG):
                pbase = (h % 2) * 64
                ft = h // 2
                for qb in range(NQB):
                    qsl = slice(qb * 512, (qb + 1) * 512)
                    probs = []
                    for kt in range(NST):
                        sc = psA.tile([128, 512], F32, tag="psA")
                        nc.tensor.matmul(
                            sc[:],
                            ck[pbase:pbase + 64, ft, kt * 128:(kt + 1) * 128],
                            cq[pbase:pbase + 64, ft, qsl],
                            start=True,
                            stop=True,
                        )
                        pr = probs_pool.tile([128, 512], BF16, tag="probs")
                        nc.scalar.activation(
                            out=pr[:], in_=sc[:],
                            func=mybir.ActivationFunctionType.Exp,
                        )
                        probs.append(pr)

                    av = psAV.tile([HD + 1, 512], F32, tag="psAV")
                    for kt in range(NST):
                        nc.tensor.matmul(
                            av[:],
                            vt[:, kt, h, :],
                            probs[kt][:],
                            start=(kt == 0),
                            stop=(kt == NST - 1),
                        )

                    num = evict_pool.tile([64, 512], F32, tag="num")
                    nc.vector.tensor_copy(out=num[:], in_=av[0:HD, :])
                    den = evict_pool.tile([1, 512], F32, tag="den")
                    nc.vector.tensor_copy(out=den[:], in_=av[HD:HD + 1, :])
                    rec = evict_pool.tile([1, 512], F32, tag="rec")
                    nc.vector.reciprocal_approx_fast(out=rec[:], in_=den[:])
                    bc = psB.tile([64, 512], F32, tag="psB")
                    nc.tensor.matmul(bc[:], ones[:], rec[:], start=True, stop=True)
                    if debug_dumps and h == 0 and qb == 0:
                        st_av = evict_pool.tile([HD + 1, 512], F32, tag="dbgav")
                        nc.vector.tensor_copy(out=st_av[:], in_=av[:])
                        st_bc = evict_pool.tile([HD, 512], F32, tag="dbgbc")
                        nc.vector.tensor_copy(out=st_bc[:], in_=bc[:])
                        nc.sync.dma_start(out=dbg["d_pr"][:], in_=probs[0][:])
                        nc.sync.dma_start(out=dbg["d_av"][:], in_=st_av[:])
                        nc.sync.dma_start(out=dbg["d_rec"][:], in_=rec[:])
                        nc.sync.dma_start(out=dbg["d_bc"][:], in_=st_bc[:])
                        nc.sync.dma_start(out=dbg["d_cq"][:], in_=cq[:, 0, :])
                        nc.sync.dma_start(out=dbg["d_ck"][:], in_=ck[:, 0, :])
                        nc.sync.dma_start(out=dbg["d_v"][:], in_=vt[:, 0, 0, :])
                    nc.vector.tensor_tensor(
                        out=yt[pbase:pbase + 64, ft, qsl],
                        in0=num[:],
                        in1=bc[:],
                        op=mybir.AluOpType.mult,
                    )

            # ---- output DMAs ----
            for ft in range(NFT):
                nc.sync.dma_start(out=yT[ft * 128:(ft + 1) * 128, :], in_=yt[:, ft, :])

    nc.finalize()
    return nc


def _get_nc():
    if "nc" not in _CACHE:
        _CACHE["nc"] = _build()
    return _CACHE["nc"]


def kernel(x, tokens, Wq, bq, Wk, bk, Wv, bv):
    x = np.asarray(x, dtype=np.float32)
    tokens = np.asarray(tokens, dtype=np.float32)
    Wq = np.asarray(Wq, dtype=np.float32)
    Wk = np.asarray(Wk, dtype=np.float32)
    Wv = np.asarray(Wv, dtype=np.float32)
    bq = np.asarray(bq, dtype=np.float32)
    bk = np.asarray(bk, dtype=np.float32)
    bv = np.asarray(bv, dtype=np.float32)

    bf16 = ml_dtypes.bfloat16
    in_maps = []
    for c in range(NCORES):
        b, g = divmod(c, 2)
        rows = slice(g * FPG, (g + 1) * FPG)
        tq = tokens[b, 0] @ Wq[rows].T + 2.0 * bq[rows]   # [512]
        tk = tokens[b, 0] @ Wk[rows].T + 2.0 * bk[rows]
        in_maps.append({
            "xT": np.ascontiguousarray(x[b].T).astype(bf16),
            "wqT": np.ascontiguousarray(Wq[rows].T).astype(bf16),
            "wkT": np.ascontiguousarray(Wk[rows].T).astype(bf16),
            "wvT": np.ascontiguousarray(Wv[rows].T).astype(bf16),
            "qadd": np.ascontiguousarray((tq / 8.0).reshape(NFT, 128).T).astype(np.float32),
            "kadd": np.ascontiguousarray(tk.reshape(NFT, 128).T).astype(np.float32),
        })

    nc = _get_nc()
    trace = bool(int(os.environ.get("KERNEL_TRACE", "0")))
    res = run_bass_kernel_spmd(nc, in_maps, core_ids=list(range(NCORES)), trace=trace)
    if trace:
        _CACHE["last_results"] = res

    y = np.empty((B, S, D), dtype=np.float32)
    for c in range(NCORES):
        b, g = divmod(c, 2)
        y[b, :, g * FPG:(g + 1) * FPG] = res.results[c]["yT"].T
    y += bv[None, None, :]
    return y


# revision 14
# speedup vs baseline: 1.5062x; 1.5062x over previous
"""Guide-token attention kernel for Trainium2 (8 NeuronCores).

Module: y[b] = softmax(((Q+tQ) @ (K+tK)^T)/sqrt(hd)) @ V  per head, where
  Q = x @ Wq^T + bq, K = x @ Wk^T + bk, V = x @ Wv^T + bv,
  tQ/tK are projections of a per-batch guide token (broadcast over seq).

Shapes: x [4, 1024, 1024], tokens [4, 1, 1024], W* [1024, 1024], b* [1024].
H=16 heads, hd=64.

Sharding: 8 cores = 4 batches x 2 head-groups (8 heads each); weights
column-sharded per head group; each core sees one batch -> no cross-core
communication.

Layout (PE contracts over the partition axis; no on-chip transposes):
  - host pre-transposes x[b] -> xT [D, S] and W slices -> wT [D, 512] (bf16),
    and precomputes the tiny guide-token adds (tq + 2*bq etc.).
  - QT/KT computed transposed [feat, S]; V computed natural [S, feat].
  - scores computed directly transposed per head: sT[k, q] = cK @ cQ^T
    (lhsT = cKT slice, rhs = cQT slice, contraction = hd = 64).
  - exp on ScalarE over two-bank PSUM tiles [128, 2, 512] (amortizes the
    ~352-cycle ACTIVATE overhead), writing bf16 probs. Softmax max-
    subtraction skipped: |scores| <= ~15 so exp is safely in fp32/bf16 range.
  - AV: lhsT = V chunk [k, 64] + appended ones column (row 64 of the output
    accumulates the softmax denominator), rhs = probsT [k, q], accumulated
    over k chunks -> [65, q] PSUM.
  - normalize: denominator row -> SBUF, reciprocal (fast-approx), GpSimd
    partition_broadcast to 64 rows, one VectorE multiply -> yT [feat, S].
  - host reassembles y[b][:, cols] = yT^T and adds bv once at the end
    (softmax rows sum to 1, so y = softmax@V0 + bv exactly).

Schedule (HAM-aware): the PE instruction stream is kept dense so the clock
gate stays at 8/8. QK(ft0) runs first; the h0-h3 attention units then
interleave V and QK(ft1) matmuls as fillers between score pairs (3 filler
MMs per pair exactly consumes the 96 remaining projection MMs at the rate
ACT drains exps); the h4-h7 score units interleave with the h0-h3 AV units.
"""

import os

import numpy as np
import ml_dtypes

import concourse.bass as bass
import concourse.tile as tile
from concourse import bacc
from concourse import mybir
from concourse.bass_utils import run_bass_kernel_spmd

B = 4
S = 1024
D = 1024
H = 16
HD = 64
NCORES = 8
FPG = 512          # features per head-group (8 heads * 64)
NKC = D // 128     # contraction chunks for projections
NFT = FPG // 128   # feature tiles per group
NST = S // 128     # sequence tiles
NQB = S // 512     # 512-wide query blocks
HPG = 8            # heads per group
NPAIR = NST // 2   # kt pairs per unit

BF16 = mybir.dt.bfloat16
F32 = mybir.dt.float32

_CACHE = {}


def _build():
    nc = bacc.Bacc()

    xT = nc.declare_dram_parameter("xT", [D, S], BF16, isOutput=False)
    wqT = nc.declare_dram_parameter("wqT", [D, FPG], BF16, isOutput=False)
    wkT = nc.declare_dram_parameter("wkT", [D, FPG], BF16, isOutput=False)
    wvT = nc.declare_dram_parameter("wvT", [D, FPG], BF16, isOutput=False)
    qadd = nc.declare_dram_parameter("qadd", [128, NFT], F32, isOutput=False)
    kadd = nc.declare_dram_parameter("kadd", [128, NFT], F32, isOutput=False)
    yT = nc.declare_dram_parameter("yT", [FPG, S], F32, isOutput=True)

    with tile.TileContext(nc) as tc:
        with (
            tc.tile_pool(name="persist", bufs=1) as persist,
            tc.tile_pool(name="probs", bufs=44) as probs_pool,
            tc.tile_pool(name="norm", bufs=4) as norm_pool,
            tc.tile_pool(name="psP", bufs=2, space=bass.MemorySpace.PSUM) as psP,
            tc.tile_pool(name="psA", bufs=2, space=bass.MemorySpace.PSUM) as psA,
            tc.tile_pool(name="psAV", bufs=2, space=bass.MemorySpace.PSUM) as psAV,
        ):
            # ---- persistent SBUF tensors ----
            xt = persist.tile([128, NKC, S], BF16)
            wq = persist.tile([128, NKC, FPG], BF16)
            wk = persist.tile([128, NKC, FPG], BF16)
            wv = persist.tile([128, NKC, FPG], BF16)
            qa = persist.tile([128, NFT], F32)
            ka = persist.tile([128, NFT], F32)
            cq = persist.tile([128, NFT, S], BF16)          # cQT/8  [feat, S]
            ck = persist.tile([128, NFT, S], BF16)          # cKT    [feat, S]
            vt = persist.tile([128, NST, HPG, HD + 1], BF16)  # V' + ones col
            yt = persist.tile([128, NFT, S], F32)           # yT [feat, S]

            # ---- input DMAs (wq/x first: QK ft0 starts the kernel) ----
            for kc in range(NKC):
                nc.sync.dma_start(out=wq[:, kc, :], in_=wqT[kc * 128:(kc + 1) * 128, :])
                nc.sync.dma_start(out=xt[:, kc, :], in_=xT[kc * 128:(kc + 1) * 128, :])
            nc.sync.dma_start(out=qa[:], in_=qadd[:])
            nc.sync.dma_start(out=ka[:], in_=kadd[:])
            for kc in range(NKC):
                nc.sync.dma_start(out=wk[:, kc, :], in_=wkT[kc * 128:(kc + 1) * 128, :])
            for kc in range(NKC):
                nc.sync.dma_start(out=wv[:, kc, :], in_=wvT[kc * 128:(kc + 1) * 128, :])

            nc.vector.memset(vt[:, :, :, HD:HD + 1], 1.0)

            # ---- projection building blocks ----
            def qk_group(which, ft, sb):
                """QT/KT [feat tile, S block] accumulated over D chunks,
                evicted to bf16 with the guide-token add (+1/8 scale for Q)."""
                w_sb, add_sb, scale, dst = (
                    (wq, qa, 0.125, cq) if which == "q" else (wk, ka, 1.0, ck)
                )
                acc = psP.tile([128, 512], F32, tag="psP")
                for kc in range(NKC):
                    yield lambda kc=kc, acc=acc: nc.tensor.matmul(
                        acc[:],
                        w_sb[:, kc, ft * 128:(ft + 1) * 128],
                        xt[:, kc, sb * 512:(sb + 1) * 512],
                        start=(kc == 0),
                        stop=(kc == NKC - 1),
                    )
                yield lambda acc=acc: nc.vector.tensor_scalar(
                    out=dst[:, ft, sb * 512:(sb + 1) * 512],
                    in0=acc[:],
                    scalar1=scale,
                    scalar2=add_sb[:, ft:ft + 1],
                    op0=mybir.AluOpType.mult,
                    op1=mybir.AluOpType.add,
                )

            def v_group(st):
                """V [S tile, feat] natural layout, strided into vt."""
                acc = psP.tile([128, 512], F32, tag="psP")
                for kc in range(NKC):
                    yield lambda kc=kc, acc=acc: nc.tensor.matmul(
                        acc[:],
                        xt[:, kc, st * 128:(st + 1) * 128],
                        wv[:, kc, :],
                        start=(kc == 0),
                        stop=(kc == NKC - 1),
                    )
                yield lambda acc=acc: nc.vector.tensor_copy(
                    out=vt[:, st, :, 0:HD], in_=acc[:]
                )

            def run(gen):
                for op in gen:
                    op()

            # filler stream: QK ft2/ft3 + all of V (96 MMs + evictions)
            def filler_stream():
                for which in ("q", "k"):
                    for ft in (2, 3):
                        for sb in range(NQB):
                            yield from qk_group(which, ft, sb)
                for st in range(NST):
                    yield from v_group(st)

            # ---- attention building blocks ----
            def unit_scores(h, qb, filler=None):
                """8 score MMs (paired into 2-bank PSUM tiles) + exp pairs.
                Returns the 4 probs pair-tiles. Pulls 3 filler ops per pair."""
                pbase = (h % 2) * 64
                ft = h // 2
                qsl = slice(qb * 512, (qb + 1) * 512)
                pairs = []
                for p in range(NPAIR):
                    sc = psA.tile([128, 2, 512], F32, tag="psA")
                    for j in range(2):
                        kt = 2 * p + j
                        nc.tensor.matmul(
                            sc[:, j, :],
                            ck[pbase:pbase + 64, ft, kt * 128:(kt + 1) * 128],
                            cq[pbase:pbase + 64, ft, qsl],
                            start=True,
                            stop=True,
                        )
                    pr = probs_pool.tile([128, 2, 512], BF16, tag="probs")
                    nc.scalar.activation(
                        out=pr[:], in_=sc[:],
                        func=mybir.ActivationFunctionType.Exp,
                    )
                    pairs.append(pr)
                    if filler is not None:
                        for _ in range(3):
                            op = next(filler, None)
                            if op is not None:
                                op()
                return pairs

            def unit_av(h, qb, pairs):
                """AV accumulation + softmax normalization -> yt slice."""
                pbase = (h % 2) * 64
                ft = h // 2
                qsl = slice(qb * 512, (qb + 1) * 512)
                av = psAV.tile([HD + 1, 512], F32, tag="psAV")
                for kt in range(NST):
                    nc.tensor.matmul(
                        av[:],
                        vt[:, kt, h, :],
                        pairs[kt // 2][:, kt % 2, :],
                        start=(kt == 0),
                        stop=(kt == NST - 1),
                    )
                den = norm_pool.tile([1, 512], F32, tag="den")
                nc.vector.tensor_copy(out=den[:], in_=av[HD:HD + 1, :])
                rec = norm_pool.tile([1, 512], F32, tag="rec")
                nc.vector.reciprocal_approx_fast(out=rec[:], in_=den[:])
                recb = norm_pool.tile([HD, 512], F32, tag="recb")
                nc.gpsimd.partition_broadcast(recb[:], rec[:])
                nc.vector.tensor_tensor(
                    out=yt[pbase:pbase + 64, ft, qsl],
                    in0=av[0:HD, :],
                    in1=recb[:],
                    op=mybir.AluOpType.mult,
                )

            # ---- schedule ----
            # Phase 1: QK ft0/ft1 dense (heads 0-3 depend only on these).
            for which in ("q", "k"):
                for ft in (0, 1):
                    for sb in range(NQB):
                        run(qk_group(which, ft, sb))

            units = [(h, qb) for h in range(HPG) for qb in range(NQB)]
            early, late = units[:8], units[8:]

            # Phase 2: early score units with projection fillers.
            filler = filler_stream()
            pairs_of = {}
            for h, qb in early:
                pairs_of[(h, qb)] = unit_scores(h, qb, filler=filler)
            for op in filler:   # drain any remainder (V must precede AV)
                op()

            # Phase 3: early AV interleaved with late score units.
            for i, (h, qb) in enumerate(late):
                unit_av(*early[i], pairs_of.pop(early[i]))
                pairs_of[(h, qb)] = unit_scores(h, qb)

            # Phase 4: late AV units.
            for h, qb in late:
                unit_av(h, qb, pairs_of.pop((h, qb)))

            # ---- output DMAs ----
            for ft in range(NFT):
                nc.sync.dma_start(out=yT[ft * 128:(ft + 1) * 128, :], in_=yt[:, ft, :])

    nc.finalize()
    return nc


def _get_nc():
    if "nc" not in _CACHE:
        _CACHE["nc"] = _build()
    return _CACHE["nc"]


def kernel(x, tokens, Wq, bq, Wk, bk, Wv, bv):
    x = np.asarray(x, dtype=np.float32)
    tokens = np.asarray(tokens, dtype=np.float32)
    Wq = np.asarray(Wq, dtype=np.float32)
    Wk = np.asarray(Wk, dtype=np.float32)
    Wv = np.asarray(Wv, dtype=np.float32)
    bq = np.asarray(bq, dtype=np.float32)
    bk = np.asarray(bk, dtype=np.float32)
    bv = np.asarray(bv, dtype=np.float32)

    bf16 = ml_dtypes.bfloat16
    in_maps = []
    for c in range(NCORES):
        b, g = divmod(c, 2)
        rows = slice(g * FPG, (g + 1) * FPG)
        tq = tokens[b, 0] @ Wq[rows].T + 2.0 * bq[rows]   # [512]
        tk = tokens[b, 0] @ Wk[rows].T + 2.0 * bk[rows]
        in_maps.append({
            "xT": np.ascontiguousarray(x[b].T).astype(bf16),
            "wqT": np.ascontiguousarray(Wq[rows].T).astype(bf16),
            "wkT": np.ascontiguousarray(Wk[rows].T).astype(bf16),
            "wvT": np.ascontiguousarray(Wv[rows].T).astype(bf16),
            "qadd": np.ascontiguousarray((tq / 8.0).reshape(NFT, 128).T).astype(np.float32),
            "kadd": np.ascontiguousarray(tk.reshape(NFT, 128).T).astype(np.float32),
        })

    nc = _get_nc()
    trace = bool(int(os.environ.get("KERNEL_TRACE", "0")))
    res = run_bass_kernel_spmd(nc, in_maps, core_ids=list(range(NCORES)), trace=trace)
    if trace:
        _CACHE["last_results"] = res

    y = np.empty((B, S, D), dtype=np.float32)
    for c in range(NCORES):
        b, g = divmod(c, 2)
        y[b, :, g * FPG:(g + 1) * FPG] = res.results[c]["yT"].T
    y += bv[None, None, :]
    return y


# revision 17
# speedup vs baseline: 1.5070x; 1.0006x over previous
"""Guide-token attention kernel for Trainium2 (8 NeuronCores).

Module: y[b] = softmax(((Q+tQ) @ (K+tK)^T)/sqrt(hd)) @ V  per head, where
  Q = x @ Wq^T + bq, K = x @ Wk^T + bk, V = x @ Wv^T + bv,
  tQ/tK are projections of a per-batch guide token (broadcast over seq).

Shapes: x [4, 1024, 1024], tokens [4, 1, 1024], W* [1024, 1024], b* [1024].
H=16 heads, hd=64.

Sharding: 8 cores = 4 batches x 2 head-groups (8 heads each); weights
column-sharded per head group; each core sees one batch -> no cross-core
communication.

Layout (PE contracts over the partition axis; no on-chip transposes):
  - host pre-transposes x[b] -> xT [D, S] and W slices -> wT [D, 512] (bf16),
    and precomputes the tiny guide-token adds (tq + 2*bq etc.).
  - QT/KT computed transposed [feat, S]; V computed natural [S, feat].
  - scores computed directly transposed per head: sT[k, q] = cK @ cQ^T
    (lhsT = cKT slice, rhs = cQT slice, contraction = hd = 64).
  - exp on ScalarE over two-bank PSUM tiles [128, 2, 512] (amortizes the
    ~352-cycle ACTIVATE overhead), writing bf16 probs. Softmax max-
    subtraction skipped: |scores| <= ~15 so exp is safely in fp32/bf16 range.
  - AV: lhsT = V chunk [k, 64] + appended ones column (row 64 of the output
    accumulates the softmax denominator), rhs = probsT [k, q], accumulated
    over k chunks -> [65, q] PSUM.
  - normalize: denominator row -> SBUF, reciprocal (fast-approx), GpSimd
    partition_broadcast to 64 rows, one VectorE multiply -> yT [feat, S].
  - host reassembles y[b][:, cols] = yT^T and adds bv once at the end
    (softmax rows sum to 1, so y = softmax@V0 + bv exactly).

Schedule (HAM-aware): the PE instruction stream is kept dense so the clock
gate stays at 8/8. QK(ft0) runs first; the h0-h3 attention units then
interleave V and QK(ft1) matmuls as fillers between score pairs (3 filler
MMs per pair exactly consumes the 96 remaining projection MMs at the rate
ACT drains exps); the h4-h7 score units interleave with the h0-h3 AV units.
"""

import os

import numpy as np
import ml_dtypes

import concourse.bass as bass
import concourse.tile as tile
from concourse import bacc
from concourse import mybir
from concourse.bass_utils import run_bass_kernel_spmd

B = 4
S = 1024
D = 1024
H = 16
HD = 64
NCORES = 8
FPG = 512          # features per head-group (8 heads * 64)
NKC = D // 128     # contraction chunks for projections
NFT = FPG // 128   # feature tiles per group
NST = S // 128     # sequence tiles
NQB = S // 512     # 512-wide query blocks
HPG = 8            # heads per group
NPAIR = NST // 2   # kt pairs per unit

BF16 = mybir.dt.bfloat16
F32 = mybir.dt.float32

_CACHE = {}


def _build():
    nc = bacc.Bacc()

    xT = nc.declare_dram_parameter("xT", [D, S], BF16, isOutput=False)
    wqT = nc.declare_dram_parameter("wqT", [D, FPG], BF16, isOutput=False)
    wkT = nc.declare_dram_parameter("wkT", [D, FPG], BF16, isOutput=False)
    wvT = nc.declare_dram_parameter("wvT", [D, FPG], BF16, isOutput=False)
    qadd = nc.declare_dram_parameter("qadd", [128, NFT], F32, isOutput=False)
    kadd = nc.declare_dram_parameter("kadd", [128, NFT], F32, isOutput=False)
    yT = nc.declare_dram_parameter("yT", [FPG, S], F32, isOutput=True)

    with tile.TileContext(nc) as tc:
        with (
            tc.tile_pool(name="persist", bufs=1) as persist,
            tc.tile_pool(name="probs", bufs=44) as probs_pool,
            tc.tile_pool(name="norm", bufs=4) as norm_pool,
            tc.tile_pool(name="psP", bufs=2, space=bass.MemorySpace.PSUM) as psP,
            tc.tile_pool(name="psA", bufs=2, space=bass.MemorySpace.PSUM) as psA,
            tc.tile_pool(name="psAV", bufs=2, space=bass.MemorySpace.PSUM) as psAV,
        ):
            # ---- persistent SBUF tensors ----
            xt = persist.tile([128, NKC, S], BF16)
            wq = persist.tile([128, NKC, FPG], BF16)
            wk = persist.tile([128, NKC, FPG], BF16)
            wv = persist.tile([128, NKC, FPG], BF16)
            qa = persist.tile([128, NFT], F32)
            ka = persist.tile([128, NFT], F32)
            cq = persist.tile([128, NFT, S], BF16)          # cQT/8  [feat, S]
            ck = persist.tile([128, NFT, S], BF16)          # cKT    [feat, S]
            vt = persist.tile([128, NST, HPG, HD + 1], BF16)  # V' + ones col
            yt = persist.tile([128, NFT, S], F32)           # yT [feat, S]

            # ---- input DMAs (wq/x first: QK ft0 starts the kernel) ----
            nc.sync.dma_start(out=qa[:], in_=qadd[:])
            nc.sync.dma_start(out=ka[:], in_=kadd[:])
            for kc in range(NKC):
                nc.sync.dma_start(out=wq[:, kc, :], in_=wqT[kc * 128:(kc + 1) * 128, :])
                nc.sync.dma_start(out=xt[:, kc, :], in_=xT[kc * 128:(kc + 1) * 128, :])
            for kc in range(NKC):
                nc.sync.dma_start(out=wk[:, kc, :], in_=wkT[kc * 128:(kc + 1) * 128, :])
            for kc in range(NKC):
                nc.sync.dma_start(out=wv[:, kc, :], in_=wvT[kc * 128:(kc + 1) * 128, :])

            nc.vector.memset(vt[:, :, :, HD:HD + 1], 1.0)

            # ---- HAM pre-warm: dummy matmuls while input DMAs stream ----
            # The PE clock gate needs ~3.4us of sustained activity to go
            # 8/8; burn the DMA head (~10us) on throwaway matmuls so the
            # real projections start at full clock.
            wrm = persist.tile([128, 512], BF16)
            nc.vector.memset(wrm[:], 0.0)
            wacc = psAV.tile([128, 512], F32, tag="psAV")
            for _ in range(30):
                nc.tensor.matmul(
                    wacc[:], wrm[:, 0:128], wrm[:], start=True, stop=True
                )

            # ---- projection building blocks ----
            def qk_group(which, ft, sb):
                """QT/KT [feat tile, S block] accumulated over D chunks,
                evicted to bf16 with the guide-token add (+1/8 scale for Q)."""
                w_sb, add_sb, scale, dst = (
                    (wq, qa, 0.125, cq) if which == "q" else (wk, ka, 1.0, ck)
                )
                acc = psP.tile([128, 512], F32, tag="psP")
                for kc in range(NKC):
                    yield lambda kc=kc, acc=acc: nc.tensor.matmul(
                        acc[:],
                        w_sb[:, kc, ft * 128:(ft + 1) * 128],
                        xt[:, kc, sb * 512:(sb + 1) * 512],
                        start=(kc == 0),
                        stop=(kc == NKC - 1),
                    )
                yield lambda acc=acc: nc.vector.tensor_scalar(
                    out=dst[:, ft, sb * 512:(sb + 1) * 512],
                    in0=acc[:],
                    scalar1=scale,
                    scalar2=add_sb[:, ft:ft + 1],
                    op0=mybir.AluOpType.mult,
                    op1=mybir.AluOpType.add,
                )

            def v_group(st):
                """V [S tile, feat] natural layout, strided into vt."""
                acc = psP.tile([128, 512], F32, tag="psP")
                for kc in range(NKC):
                    yield lambda kc=kc, acc=acc: nc.tensor.matmul(
                        acc[:],
                        xt[:, kc, st * 128:(st + 1) * 128],
                        wv[:, kc, :],
                        start=(kc == 0),
                        stop=(kc == NKC - 1),
                    )
                yield lambda acc=acc: nc.vector.tensor_copy(
                    out=vt[:, st, :, 0:HD], in_=acc[:]
                )

            def run(gen):
                for op in gen:
                    op()

            # filler stream: QK ft2/ft3 + all of V (96 MMs + evictions)
            def filler_stream():
                for which in ("q", "k"):
                    for ft in (2, 3):
                        for sb in range(NQB):
                            yield from qk_group(which, ft, sb)
                for st in range(NST):
                    yield from v_group(st)

            # ---- attention building blocks ----
            def unit_scores(h, qb, filler=None):
                """8 score MMs (paired into 2-bank PSUM tiles) + exp pairs.
                Returns the 4 probs pair-tiles. Pulls 3 filler ops per pair."""
                pbase = (h % 2) * 64
                ft = h // 2
                qsl = slice(qb * 512, (qb + 1) * 512)
                pairs = []
                for p in range(NPAIR):
                    sc = psA.tile([128, 2, 512], F32, tag="psA")
                    for j in range(2):
                        kt = 2 * p + j
                        nc.tensor.matmul(
                            sc[:, j, :],
                            ck[pbase:pbase + 64, ft, kt * 128:(kt + 1) * 128],
                            cq[pbase:pbase + 64, ft, qsl],
                            start=True,
                            stop=True,
                        )
                    pr = probs_pool.tile([128, 2, 512], BF16, tag="probs")
                    nc.scalar.activation(
                        out=pr[:], in_=sc[:],
                        func=mybir.ActivationFunctionType.Exp,
                    )
                    pairs.append(pr)
                    if filler is not None:
                        for _ in range(3):
                            op = next(filler, None)
                            if op is not None:
                                op()
                return pairs

            def unit_av(h, qb, pairs):
                """AV accumulation + softmax normalization -> yt slice."""
                pbase = (h % 2) * 64
                ft = h // 2
                qsl = slice(qb * 512, (qb + 1) * 512)
                av = psAV.tile([HD + 1, 512], F32, tag="psAV")
                for kt in range(NST):
                    nc.tensor.matmul(
                        av[:],
                        vt[:, kt, h, :],
                        pairs[kt // 2][:, kt % 2, :],
                        start=(kt == 0),
                        stop=(kt == NST - 1),
                    )
                den = norm_pool.tile([1, 512], F32, tag="den")
                nc.vector.tensor_copy(out=den[:], in_=av[HD:HD + 1, :])
                rec = norm_pool.tile([1, 512], F32, tag="rec")
                nc.vector.reciprocal_approx_fast(out=rec[:], in_=den[:])
                recb = norm_pool.tile([HD, 512], F32, tag="recb")
                nc.gpsimd.partition_broadcast(recb[:], rec[:])
                nc.vector.tensor_tensor(
                    out=yt[pbase:pbase + 64, ft, qsl],
                    in0=av[0:HD, :],
                    in1=recb[:],
                    op=mybir.AluOpType.mult,
                )

            # ---- schedule ----
            # Phase 1: QK ft0/ft1 dense (heads 0-3 depend only on these).
            for which in ("q", "k"):
                for ft in (0, 1):
                    for sb in range(NQB):
                        run(qk_group(which, ft, sb))

            units = [(h, qb) for h in range(HPG) for qb in range(NQB)]
            early, late = units[:8], units[8:]

            # Phase 2: early score units with projection fillers.
            filler = filler_stream()
            pairs_of = {}
            for h, qb in early:
                pairs_of[(h, qb)] = unit_scores(h, qb, filler=filler)
            for op in filler:   # drain any remainder (V must precede AV)
                op()

            # Output DMA per feature tile as soon as both its heads are done.
            done_units = set()

            def maybe_flush(h, qb):
                done_units.add((h, qb))
                ft = h // 2
                if all((2 * ft + dh, q) in done_units
                       for dh in range(2) for q in range(NQB)):
                    nc.sync.dma_start(
                        out=yT[ft * 128:(ft + 1) * 128, :], in_=yt[:, ft, :]
                    )

            # Phase 3: early AV interleaved with late score units.
            for i, (h, qb) in enumerate(late):
                unit_av(*early[i], pairs_of.pop(early[i]))
                maybe_flush(*early[i])
                pairs_of[(h, qb)] = unit_scores(h, qb)

            # Phase 4: late AV units.
            for h, qb in late:
                unit_av(h, qb, pairs_of.pop((h, qb)))
                maybe_flush(h, qb)

    nc.finalize()
    return nc


def _get_nc():
    if "nc" not in _CACHE:
        _CACHE["nc"] = _build()
    return _CACHE["nc"]


def kernel(x, tokens, Wq, bq, Wk, bk, Wv, bv):
    x = np.asarray(x, dtype=np.float32)
    tokens = np.asarray(tokens, dtype=np.float32)
    Wq = np.asarray(Wq, dtype=np.float32)
    Wk = np.asarray(Wk, dtype=np.float32)
    Wv = np.asarray(Wv, dtype=np.float32)
    bq = np.asarray(bq, dtype=np.float32)
    bk = np.asarray(bk, dtype=np.float32)
    bv = np.asarray(bv, dtype=np.float32)

    bf16 = ml_dtypes.bfloat16
    in_maps = []
    for c in range(NCORES):
        b, g = divmod(c, 2)
        rows = slice(g * FPG, (g + 1) * FPG)
        tq = tokens[b, 0] @ Wq[rows].T + 2.0 * bq[rows]   # [512]
        tk = tokens[b, 0] @ Wk[rows].T + 2.0 * bk[rows]
        in_maps.append({
            "xT": np.ascontiguousarray(x[b].T).astype(bf16),
            "wqT": np.ascontiguousarray(Wq[rows].T).astype(bf16),
            "wkT": np.ascontiguousarray(Wk[rows].T).astype(bf16),
            "wvT": np.ascontiguousarray(Wv[rows].T).astype(bf16),
            "qadd": np.ascontiguousarray((tq / 8.0).reshape(NFT, 128).T).astype(np.float32),
            "kadd": np.ascontiguousarray(tk.reshape(NFT, 128).T).astype(np.float32),
        })

    nc = _get_nc()
    trace = bool(int(os.environ.get("KERNEL_TRACE", "0")))
    res = run_bass_kernel_spmd(nc, in_maps, core_ids=list(range(NCORES)), trace=trace)
    if trace:
        _CACHE["last_results"] = res

    y = np.empty((B, S, D), dtype=np.float32)
    for c in range(NCORES):
        b, g = divmod(c, 2)
        y[b, :, g * FPG:(g + 1) * FPG] = res.results[c]["yT"].T
    y += bv[None, None, :]
    return y


# revision 18
# speedup vs baseline: 1.5581x; 1.0339x over previous
"""Guide-token attention kernel for Trainium2 (8 NeuronCores).

Module: y[b] = softmax(((Q+tQ) @ (K+tK)^T)/sqrt(hd)) @ V  per head, where
  Q = x @ Wq^T + bq, K = x @ Wk^T + bk, V = x @ Wv^T + bv,
  tQ/tK are projections of a per-batch guide token (broadcast over seq).

Shapes: x [4, 1024, 1024], tokens [4, 1, 1024], W* [1024, 1024], b* [1024].
H=16 heads, hd=64.

Sharding: 8 cores = 4 batches x 2 head-groups (8 heads each); weights
column-sharded per head group; each core sees one batch -> no cross-core
communication.

Layout (PE contracts over the partition axis; no on-chip transposes):
  - host pre-transposes x[b] -> xT [D, S] and W slices -> wT [D, 512] (bf16),
    and precomputes the tiny guide-token adds (tq + 2*bq etc.).
  - QT/KT computed transposed [feat, S]; V computed natural [S, feat].
  - scores computed directly transposed per head: sT[k, q] = cK @ cQ^T
    (lhsT = cKT slice, rhs = cQT slice, contraction = hd = 64).
  - exp on ScalarE over two-bank PSUM tiles [128, 2, 512] (amortizes the
    ~352-cycle ACTIVATE overhead), writing bf16 probs. Softmax max-
    subtraction skipped: |scores| <= ~15 so exp is safely in fp32/bf16 range.
  - AV: lhsT = V chunk [k, 64] + appended ones column (row 64 of the output
    accumulates the softmax denominator), rhs = probsT [k, q], accumulated
    over k chunks -> [65, q] PSUM.
  - normalize: denominator row -> SBUF, reciprocal (fast-approx), GpSimd
    partition_broadcast to 64 rows, one VectorE multiply -> yT [feat, S].
  - host reassembles y[b][:, cols] = yT^T and adds bv once at the end
    (softmax rows sum to 1, so y = softmax@V0 + bv exactly).

Schedule (HAM-aware): the PE instruction stream is kept dense so the clock
gate stays at 8/8. QK(ft0) runs first; the h0-h3 attention units then
interleave V and QK(ft1) matmuls as fillers between score pairs (3 filler
MMs per pair exactly consumes the 96 remaining projection MMs at the rate
ACT drains exps); the h4-h7 score units interleave with the h0-h3 AV units.
"""

import os

import numpy as np
import ml_dtypes

import concourse.bass as bass
import concourse.tile as tile
from concourse import bacc
from concourse import mybir
from concourse.bass_utils import run_bass_kernel_spmd

B = 4
S = 1024
D = 1024
H = 16
HD = 64
NCORES = 8
FPG = 512          # features per head-group (8 heads * 64)
NKC = D // 128     # contraction chunks for projections
NFT = FPG // 128   # feature tiles per group
NST = S // 128     # sequence tiles
NQB = S // 512     # 512-wide query blocks
HPG = 8            # heads per group
NPAIR = NST // 2   # kt pairs per unit

BF16 = mybir.dt.bfloat16
F32 = mybir.dt.float32

_CACHE = {}


def _build():
    nc = bacc.Bacc()

    xT = nc.declare_dram_parameter("xT", [D, S], BF16, isOutput=False)
    wqT = nc.declare_dram_parameter("wqT", [D, FPG], BF16, isOutput=False)
    wkT = nc.declare_dram_parameter("wkT", [D, FPG], BF16, isOutput=False)
    wvT = nc.declare_dram_parameter("wvT", [D, FPG], BF16, isOutput=False)
    qadd = nc.declare_dram_parameter("qadd", [128, NFT], F32, isOutput=False)
    kadd = nc.declare_dram_parameter("kadd", [128, NFT], F32, isOutput=False)
    yT = nc.declare_dram_parameter("yT", [FPG, S], F32, isOutput=True)

    with tile.TileContext(nc) as tc:
        with (
            tc.tile_pool(name="persist", bufs=1) as persist,
            tc.tile_pool(name="probs", bufs=44) as probs_pool,
            tc.tile_pool(name="norm", bufs=4) as norm_pool,
            tc.tile_pool(name="psP", bufs=2, space=bass.MemorySpace.PSUM) as psP,
            tc.tile_pool(name="psA", bufs=2, space=bass.MemorySpace.PSUM) as psA,
            tc.tile_pool(name="psAV", bufs=2, space=bass.MemorySpace.PSUM) as psAV,
        ):
            # ---- persistent SBUF tensors ----
            xt = persist.tile([128, NKC, S], BF16)
            wq = persist.tile([128, NKC, FPG], BF16)
            wk = persist.tile([128, NKC, FPG], BF16)
            wv = persist.tile([128, NKC, FPG], BF16)
            qa = persist.tile([128, NFT], F32)
            ka = persist.tile([128, NFT], F32)
            cq = persist.tile([128, NFT, S], BF16)          # cQT/8  [feat, S]
            ck = persist.tile([128, NFT, S], BF16)          # cKT    [feat, S]
            vt = persist.tile([128, NST, HPG, HD + 1], BF16)  # V' + ones col
            yt = persist.tile([128, NFT, S], F32)           # yT [feat, S]

            # ---- input DMAs (wq/x first: QK ft0 starts the kernel) ----
            nc.sync.dma_start(out=qa[:], in_=qadd[:])
            nc.sync.dma_start(out=ka[:], in_=kadd[:])
            for kc in range(NKC):
                nc.sync.dma_start(out=wq[:, kc, :], in_=wqT[kc * 128:(kc + 1) * 128, :])
                nc.sync.dma_start(out=xt[:, kc, :], in_=xT[kc * 128:(kc + 1) * 128, :])
            for kc in range(NKC):
                nc.sync.dma_start(out=wk[:, kc, :], in_=wkT[kc * 128:(kc + 1) * 128, :])
            for kc in range(NKC):
                nc.sync.dma_start(out=wv[:, kc, :], in_=wvT[kc * 128:(kc + 1) * 128, :])

            nc.vector.memset(vt[:, :, :, HD:HD + 1], 1.0)

            # ---- HAM pre-warm: dummy matmuls while input DMAs stream ----
            # The PE clock gate needs ~3.4us of sustained activity to go
            # 8/8; burn the DMA head (~10us) on throwaway matmuls so the
            # real projections start at full clock.
            wrm = persist.tile([128, 512], BF16)
            nc.gpsimd.memset(wrm[:], 0.0)
            wacc = psAV.tile([128, 512], F32, tag="psAV")
            for _ in range(16):
                nc.tensor.matmul(
                    wacc[:], wrm[:, 0:128], wrm[:], start=True, stop=True
                )

            # ---- projection building blocks ----
            def qk_group(which, ft, sb):
                """QT/KT [feat tile, S block] accumulated over D chunks,
                evicted to bf16 with the guide-token add (+1/8 scale for Q)."""
                w_sb, add_sb, scale, dst = (
                    (wq, qa, 0.125, cq) if which == "q" else (wk, ka, 1.0, ck)
                )
                acc = psP.tile([128, 512], F32, tag="psP")
                for kc in range(NKC):
                    yield lambda kc=kc, acc=acc: nc.tensor.matmul(
                        acc[:],
                        w_sb[:, kc, ft * 128:(ft + 1) * 128],
                        xt[:, kc, sb * 512:(sb + 1) * 512],
                        start=(kc == 0),
                        stop=(kc == NKC - 1),
                    )
                yield lambda acc=acc: nc.vector.tensor_scalar(
                    out=dst[:, ft, sb * 512:(sb + 1) * 512],
                    in0=acc[:],
                    scalar1=scale,
                    scalar2=add_sb[:, ft:ft + 1],
                    op0=mybir.AluOpType.mult,
                    op1=mybir.AluOpType.add,
                )

            def v_group(st):
                """V [S tile, feat] natural layout, strided into vt."""
                acc = psP.tile([128, 512], F32, tag="psP")
                for kc in range(NKC):
                    yield lambda kc=kc, acc=acc: nc.tensor.matmul(
                        acc[:],
                        xt[:, kc, st * 128:(st + 1) * 128],
                        wv[:, kc, :],
                        start=(kc == 0),
                        stop=(kc == NKC - 1),
                    )
                yield lambda acc=acc: nc.vector.tensor_copy(
                    out=vt[:, st, :, 0:HD], in_=acc[:]
                )

            def run(gen):
                for op in gen:
                    op()

            # filler stream: QK ft2/ft3 + all of V (96 MMs + evictions)
            def filler_stream():
                for which in ("q", "k"):
                    for ft in (2, 3):
                        for sb in range(NQB):
                            yield from qk_group(which, ft, sb)
                for st in range(NST):
                    yield from v_group(st)

            # ---- attention building blocks ----
            def unit_scores(hp, qb, filler=None):
                """Score MMs for head pair (2hp, 2hp+1), one 512-wide query
                block. The two heads' operands live on partitions 0-63 /
                64-127 -> different PE row groups, so their K=64 matmuls run
                concurrently (row tiling). exp pairs on ScalarE -> bf16
                probs. Pulls filler ops to keep the PE stream dense."""
                ft = hp
                qsl = slice(qb * 512, (qb + 1) * 512)
                pairsA, pairsB = [], []
                for p in range(NPAIR):
                    scA = psA.tile([128, 2, 512], F32, tag="psA")
                    scB = psA.tile([128, 2, 512], F32, tag="psA")
                    for j in range(2):
                        kt = 2 * p + j
                        ksl = slice(kt * 128, (kt + 1) * 128)
                        nc.tensor.matmul(
                            scA[:, j, :], ck[0:64, ft, ksl], cq[0:64, ft, qsl],
                            start=True, stop=True,
                        )
                        nc.tensor.matmul(
                            scB[:, j, :], ck[64:128, ft, ksl], cq[64:128, ft, qsl],
                            start=True, stop=True,
                        )
                    prA = probs_pool.tile([128, 2, 512], BF16, tag="probs")
                    nc.scalar.activation(
                        out=prA[:], in_=scA[:],
                        func=mybir.ActivationFunctionType.Exp,
                    )
                    prB = probs_pool.tile([128, 2, 512], BF16, tag="probs")
                    nc.scalar.activation(
                        out=prB[:], in_=scB[:],
                        func=mybir.ActivationFunctionType.Exp,
                    )
                    pairsA.append(prA)
                    pairsB.append(prB)
                    if filler is not None:
                        for _ in range(8):
                            op = next(filler, None)
                            if op is not None:
                                op()
                return pairsA, pairsB

            def head_av(h, qb, pairs):
                """AV accumulation + softmax normalization -> yt slice."""
                pbase = (h % 2) * 64
                ft = h // 2
                qsl = slice(qb * 512, (qb + 1) * 512)
                av = psAV.tile([HD + 1, 512], F32, tag="psAV")
                for kt in range(NST):
                    nc.tensor.matmul(
                        av[:],
                        vt[:, kt, h, :],
                        pairs[kt // 2][:, kt % 2, :],
                        start=(kt == 0),
                        stop=(kt == NST - 1),
                    )
                den = norm_pool.tile([1, 512], F32, tag="den")
                nc.vector.tensor_copy(out=den[:], in_=av[HD:HD + 1, :])
                rec = norm_pool.tile([1, 512], F32, tag="rec")
                nc.vector.reciprocal_approx_fast(out=rec[:], in_=den[:])
                recb = norm_pool.tile([HD, 512], F32, tag="recb")
                nc.gpsimd.partition_broadcast(recb[:], rec[:])
                nc.vector.tensor_tensor(
                    out=yt[pbase:pbase + 64, ft, qsl],
                    in0=av[0:HD, :],
                    in1=recb[:],
                    op=mybir.AluOpType.mult,
                )

            def unit_av(hp, qb, pr):
                head_av(2 * hp, qb, pr[0])
                head_av(2 * hp + 1, qb, pr[1])

            # ---- schedule ----
            # Phase 1: QK ft0/ft1 dense (head pairs 0-1 depend only on these).
            for which in ("q", "k"):
                for ft in (0, 1):
                    for sb in range(NQB):
                        run(qk_group(which, ft, sb))

            units = [(hp, qb) for hp in range(HPG // 2) for qb in range(NQB)]
            early, late = units[:4], units[4:]

            # Phase 2: early score units with projection fillers.
            filler = filler_stream()
            pairs_of = {}
            for hp, qb in early:
                pairs_of[(hp, qb)] = unit_scores(hp, qb, filler=filler)
            for op in filler:   # drain any remainder (V must precede AV)
                op()

            # Output DMA per feature tile as soon as both its units are done.
            done_units = set()

            def maybe_flush(hp, qb):
                done_units.add((hp, qb))
                if all((hp, q) in done_units for q in range(NQB)):
                    nc.sync.dma_start(
                        out=yT[hp * 128:(hp + 1) * 128, :], in_=yt[:, hp, :]
                    )

            # Phase 3: early AV interleaved with late score units.
            for i, (hp, qb) in enumerate(late):
                unit_av(*early[i], pairs_of.pop(early[i]))
                maybe_flush(*early[i])
                pairs_of[(hp, qb)] = unit_scores(hp, qb)

            # Phase 4: late AV units.
            for hp, qb in late:
                unit_av(hp, qb, pairs_of.pop((hp, qb)))
                maybe_flush(hp, qb)

    nc.finalize()
    return nc


def _get_nc():
    if "nc" not in _CACHE:
        _CACHE["nc"] = _build()
    return _CACHE["nc"]


def kernel(x, tokens, Wq, bq, Wk, bk, Wv, bv):
    x = np.asarray(x, dtype=np.float32)
    tokens = np.asarray(tokens, dtype=np.float32)
    Wq = np.asarray(Wq, dtype=np.float32)
    Wk = np.asarray(Wk, dtype=np.float32)
    Wv = np.asarray(Wv, dtype=np.float32)
    bq = np.asarray(bq, dtype=np.float32)
    bk = np.asarray(bk, dtype=np.float32)
    bv = np.asarray(bv, dtype=np.float32)

    bf16 = ml_dtypes.bfloat16
    in_maps = []
    for c in range(NCORES):
        b, g = divmod(c, 2)
        rows = slice(g * FPG, (g + 1) * FPG)
        tq = tokens[b, 0] @ Wq[rows].T + 2.0 * bq[rows]   # [512]
        tk = tokens[b, 0] @ Wk[rows].T + 2.0 * bk[rows]
        in_maps.append({
            "xT": np.ascontiguousarray(x[b].T).astype(bf16),
            "wqT": np.ascontiguousarray(Wq[rows].T).astype(bf16),
            "wkT": np.ascontiguousarray(Wk[rows].T).astype(bf16),
            "wvT": np.ascontiguousarray(Wv[rows].T).astype(bf16),
            "qadd": np.ascontiguousarray((tq / 8.0).reshape(NFT, 128).T).astype(np.float32),
            "kadd": np.ascontiguousarray(tk.reshape(NFT, 128).T).astype(np.float32),
        })

    nc = _get_nc()
    trace = bool(int(os.environ.get("KERNEL_TRACE", "0")))
    res = run_bass_kernel_spmd(nc, in_maps, core_ids=list(range(NCORES)), trace=trace)
    if trace:
        _CACHE["last_results"] = res

    y = np.empty((B, S, D), dtype=np.float32)
    for c in range(NCORES):
        b, g = divmod(c, 2)
        y[b, :, g * FPG:(g + 1) * FPG] = res.results[c]["yT"].T
    y += bv[None, None, :]
    return y
